# revision 13
# baseline (speedup 1.0000x reference)
"""Trainium2 kernel for a fuzzy-logic ConjunctionLayer forward pass.

Computes  out = 1[ (1 - x) @ 1[W > 0.5] <= 0 ]  for
x: [8192, 4096] f32, W: [4096, 2048] f32 -> out: [8192, 2048] f32.

Sharding: data-parallel over the batch dim across 8 NeuronCores
(x shard [1024, 4096] per core, W replicated), outputs concatenated.

Math: with x in [0, 1], every term (1-x)*Wb is >= 0, so
  res[m,n] <= 0  <=>  res[m,n] == 0  <=>  no k has (1-x[m,k] > 0 AND W[k,n] > .5).
Transport encodings (lossless FOR THE PREDICATE, proven + property-tested):
  - y ships as round-toward-+inf fp8e4m3 of clip(1-x, 0, 1): rounding up
    is monotone with 0 a fixed point, so y > 0 <=> 1-x > 0 <=> x < 1.
  - Wb ships as fp8e4m3 of the exact {0,1} indicator (both representable).
The device computes  acc = y.T-slab @ Wb  in f32 PSUM (every term >= 0,
no cancellation; any positive term >= 2^-9 >> f32 rounding of a <= 4096
sum, so acc == 0 exactly iff no (k) hits) and classifies out = 1[acc<=0].
fp8 enables the PE DoubleRow perf mode (2 fp8 weights per cell -> 2x
MACs/cycle, contraction 256 per matmul); the kernel is PE-bound at
~224ns per [256k x 128m x 512n] matmul.

Schedule: N is split into four 512-wide blocks (one f32 PSUM bank per
batch-chunk chain, 8 chains in flight). DMA rings: ring A (Sync HWDGE)
streams the 16 x k-pair slabs then the 8 batched output stores; ring B
(Scalar HWDGE) streams all 64 W k-pair tiles in consumption order,
issued upfront (everything is SBUF-resident: 4MB y + 8MB Wb + 2MB out).
n-blocks 0-2 run kk-outer/m-inner so chains ride the x DMA; n-block 3
runs m-outer/kk-inner so each chain's threshold epilogue and its full
[128, 2048] row store overlap the remaining chains' matmuls instead of
draining after the final matmul. Epilogues write fp8 {0,1} column
blocks into a per-chain SBUF row buffer; one 2KB-per-partition-line
store per chain. A short burst of warmup matmuls on a zeroed tile runs
during the initial DMA latency to bring the PE out of its low p-state
before the real stream begins. The first x slab is shipped as two
half-tiles so the first matmul's dependency lands in half the time.
"""

import numpy as np

import concourse.bass as bass
import concourse.mybir as mybir
import concourse.tile as tile
from concourse import bacc
from concourse.bass_utils import run_bass_kernel_spmd

BATCH, IN_DIM, N_RULES = 8192, 4096, 2048
N_CORES = 8
M_LOCAL = BATCH // N_CORES  # 1024 batch rows per core

P = 128            # SBUF partitions / matmul tile edge
NB_W = 512         # n-block width (= one f32 PSUM bank)
NB = N_RULES // NB_W        # 4 n-blocks
KT = IN_DIM // P            # 32 k-tiles
KP = KT // 2                # 16 k-pairs (DoubleRow consumes 2 per matmul)
MT = M_LOCAL // P           # 8 batch chunks per core
MH = M_LOCAL // 2           # half the batch rows (first-slab split)

F32 = mybir.dt.float32
FP8 = mybir.dt.float8e4
ALU = mybir.AluOpType
DR = mybir.MatmulPerfMode.DoubleRow

N_WARMUP = 24      # PE p-state warmup matmuls during initial DMA wait


def _body(tc: tile.TileContext, out: bass.AP, x0: bass.AP, xp: bass.AP,
          wp: bass.AP):
    nc = tc.nc
    ring_a, ring_b = nc.sync, nc.scalar  # the two HWDGE issue queues
    with (
        tc.tile_pool(name="sb", bufs=1) as sb,
        tc.tile_pool(name="ps", bufs=1, space="PSUM") as ps,
    ):
        # Warmup: ramp the PE p-state while the first slabs are in flight.
        # A small GpSimd memset (that engine is otherwise idle at start)
        # initializes the operand; the warmups then have no DMA deps and
        # start the moment the PE sequencer is up. Scratch PSUM shares
        # chain 7's bank (chain 7 starts ~2us into the real stream, long
        # after the warmups retire).
        wu = sb.tile([P, 256], FP8, tag="wu", bufs=1, name="wu")
        nc.gpsimd.memset(wu[:], 0.0)
        wu_ap = wu[:].rearrange("p (two m) -> p two m", two=2)
        wacc = ps.tile([P, NB_W], F32, tag="acc7", bufs=1, name="wacc")
        for _ in range(N_WARMUP):
            nc.tensor.matmul(wacc[:, 0:P], wu_ap, wu_ap, start=True,
                             stop=True, perf_mode=DR)

        # Resident operands. Slab 0 arrives as two half-tiles so the first
        # matmul waits on 128KB, not 256KB.
        s0h = [sb.tile([P, M_LOCAL], FP8, tag=f"s0h{h}", bufs=1,
                       name=f"s0h{h}") for h in range(2)]
        s2 = [sb.tile([P, 2 * M_LOCAL], FP8, tag=f"s{kk}", bufs=1,
                      name=f"s{kk}") for kk in range(1, KP)]
        wb2 = [[sb.tile([P, 2 * NB_W], FP8, tag=f"wb{nb}_{kk}", bufs=1,
                        name=f"wb{nb}_{kk}") for kk in range(KP)]
               for nb in range(NB)]
        obuf = [sb.tile([P, N_RULES], FP8, tag=f"o{m}", bufs=1,
                        name=f"o{m}") for m in range(MT)]

        def lhsT(kk, m):
            if kk == 0:
                v = s0h[m // 4][:].rearrange("p (two m) -> p two m", two=2)
                return v[:, :, (m % 4) * P:(m % 4 + 1) * P]
            v = s2[kk - 1][:].rearrange("p (two m) -> p two m", two=2)
            return v[:, :, m * P:(m + 1) * P]

        def rhs(nb, kk):
            return wb2[nb][kk][:].rearrange("p (two n) -> p two n", two=2)

        # Ring A: x slab stream (slab 0 split in halves so the first
        # matmul waits on 128KB, not 256KB).
        ring_a.dma_start(s0h[0][:], x0[0])
        ring_a.dma_start(s0h[1][:], x0[1])
        for kk in range(1, KP):
            ring_a.dma_start(s2[kk - 1][:], xp[kk - 1])
        # Ring B: all of W, issued upfront in consumption order; the ring
        # delivers ~128KB every ~0.6us, far ahead of the matmul stream.
        for nb in range(NB):
            for kk in range(KP):
                ring_b.dma_start(wb2[nb][kk][:], wp[kk * NB + nb])

        accs = {}

        def epilogue_m(nb, m):
            # fp8 {0,1} column block into chain m's SBUF row buffer; frees
            # the PSUM bank for the next n-block's chain m.
            nc.vector.tensor_scalar(
                obuf[m][:, nb * NB_W:(nb + 1) * NB_W], accs[m][:], 0.0, None,
                ALU.is_le)

        # n-blocks 0..2: kk-outer, m-inner (chains ride the x DMA in nb 0).
        for nb in range(NB - 1):
            for kk in range(KP):
                for m in range(MT):
                    if kk == 0:
                        accs[m] = ps.tile([P, NB_W], F32, tag=f"acc{m}",
                                          bufs=1, name=f"acc{nb}_{m}")
                    nc.tensor.matmul(accs[m][:], lhsT(kk, m), rhs(nb, kk),
                                     start=(kk == 0), stop=(kk == KP - 1),
                                     perf_mode=DR)
                    if kk == KP - 1:
                        epilogue_m(nb, m)

        def store_row(m):
            # Full output row for chain m, split across both rings: half
            # the data per ring and the two descriptor gens run in
            # parallel (2KB-line halves keep the DMA line-efficient).
            ring_a.dma_start(out[m * P:(m + 1) * P, 0:N_RULES // 2],
                             obuf[m][:, 0:N_RULES // 2])
            ring_b.dma_start(out[m * P:(m + 1) * P, N_RULES // 2:],
                             obuf[m][:, N_RULES // 2:])

        # n-block 3: m-outer, kk-inner — each chain completes consecutively
        # so its epilogue + full-row store overlap the remaining chains.
        # Chain 7 is split into two half-N accumulation chains (left in
        # chain 7's bank, right in chain 0's long-freed bank) so the
        # post-last-matmul epilogue is only [128, 256].
        nb = NB - 1
        for m in range(MT - 1):
            accs[m] = ps.tile([P, NB_W], F32, tag=f"acc{m}", bufs=1,
                              name=f"acc{nb}_{m}")
            for kk in range(KP):
                nc.tensor.matmul(accs[m][:], lhsT(kk, m), rhs(nb, kk),
                                 start=(kk == 0), stop=(kk == KP - 1),
                                 perf_mode=DR)
            epilogue_m(nb, m)
            store_row(m)
        m = MT - 1
        h = NB_W // 2
        acc_l = ps.tile([P, h], F32, tag="acc7", bufs=1, name="acc3_7l")
        acc_r = ps.tile([P, h], F32, tag="acc0", bufs=1, name="acc3_7r")
        for half, acc in ((0, acc_l), (1, acc_r)):
            for kk in range(KP):
                nc.tensor.matmul(
                    acc[:], lhsT(kk, m),
                    rhs(nb, kk)[:, :, half * h:(half + 1) * h],
                    start=(kk == 0), stop=(kk == KP - 1), perf_mode=DR)
            nc.vector.tensor_scalar(
                obuf[m][:, nb * NB_W + half * h:nb * NB_W + (half + 1) * h],
                acc[:], 0.0, None, ALU.is_le)
        store_row(m)


_NC_CACHE = {}


def _get_nc():
    if "nc" not in _NC_CACHE:
        nc = bacc.Bacc("TRN2", target_bir_lowering=False, debug=False,
                       num_devices=N_CORES)
        x0 = nc.dram_tensor("x0", [2, P, M_LOCAL], FP8, kind="ExternalInput")
        xp = nc.dram_tensor("xp", [KP - 1, P, 2 * M_LOCAL], FP8,
                            kind="ExternalInput")
        wp = nc.dram_tensor("wp", [KP * NB, P, 2 * NB_W], FP8,
                            kind="ExternalInput")
        out = nc.dram_tensor("out", [M_LOCAL, N_RULES], FP8,
                             kind="ExternalOutput")
        with tile.TileContext(nc) as tc:
            _body(tc, out.ap(), x0.ap(), xp.ap(), wp.ap())
        nc.compile()
        _NC_CACHE["nc"] = nc
    return _NC_CACHE["nc"]


def _np_fp8():
    import ml_dtypes
    return ml_dtypes.float8_e4m3


def _fp8_rtp(a: np.ndarray) -> np.ndarray:
    """Round-toward-+inf f32 -> fp8e4m3 for non-negative inputs <= 1.
    Monotone with 0 and 1 fixed points, so sign predicates are exact."""
    v = np.ascontiguousarray(a, dtype=np.float32).view(np.uint32)
    frac = v & np.uint32(0x000FFFFF)
    t = (v & ~np.uint32(0x000FFFFF)) + np.where(
        frac != 0, np.uint32(0x00100000), np.uint32(0))
    return np.minimum(t.view(np.float32), np.float32(1.0)).astype(_np_fp8())


def _permute_w(W: np.ndarray) -> np.ndarray:
    # [IN_DIM, N_RULES] -> [KP*NB, P, 2*NB_W] fp8 {0,1}: for k-pair kk,
    # n-block nb, row p holds [Wb[2kk*128+p, block], Wb[(2kk+1)*128+p, block]]
    wb = (W > 0.5).astype(_np_fp8())
    w5 = wb.reshape(KP, 2, P, NB, NB_W)              # [kk, j, p, nb, n]
    return np.ascontiguousarray(
        w5.transpose(0, 3, 2, 1, 4).reshape(KP * NB, P, 2 * NB_W))


def _permute_x(x_shard: np.ndarray):
    # [M_LOCAL, IN_DIM] -> slab-0 halves [2, P, M_LOCAL] + [KP-1, P,
    # 2*M_LOCAL] fp8: row p of slab kk holds
    # [y[:, 2kk*128+p].T, y[:, (2kk+1)*128+p].T]
    y = _fp8_rtp(np.clip(1.0 - x_shard.astype(np.float32), 0.0, 1.0))
    x4 = y.T.reshape(KP, 2, P, M_LOCAL)              # [kk, j, p, m]
    xs = np.ascontiguousarray(x4.transpose(0, 2, 1, 3).reshape(
        KP, P, 2 * M_LOCAL))
    x0 = np.ascontiguousarray(
        xs[0].reshape(P, 2, 2, MH).transpose(2, 0, 1, 3).reshape(
            2, P, M_LOCAL))                          # [half, p, (j, mh)]
    return x0, np.ascontiguousarray(xs[1:])


def kernel(x: np.ndarray, W: np.ndarray, **run_kwargs) -> np.ndarray:
    assert x.shape == (BATCH, IN_DIM) and W.shape == (IN_DIM, N_RULES)
    nc = _get_nc()
    wp = _permute_w(W)
    in_maps = []
    for c in range(N_CORES):
        x0, xs = _permute_x(x[c * M_LOCAL:(c + 1) * M_LOCAL, :])
        in_maps.append({"x0": x0, "xp": xs, "wp": wp})
    res = run_bass_kernel_spmd(nc, in_maps, core_ids=list(range(N_CORES)),
                               **run_kwargs)
    out = np.concatenate([res.results[c]["out"] for c in range(N_CORES)],
                         axis=0).astype(np.float32)  # fp8 {0,1} -> f32 exact
    if run_kwargs:
        kernel.last_results = res
    return out
